# revision 33
# baseline (speedup 1.0000x reference)
"""Trainium2 Bass kernel: masked-LSTM readout over to_dense_batch'd graphs.

v2 design (per core, SPMD over 8 cores):
 - Host: graphs sorted by length desc, dealt round-robin to 8 cores so all
   cores share one step schedule N_t. Columns within a core are length-sorted
   (prefix-active). x densified to fp16 step-major slabs [64, Wb] per step.
 - Device, per step t, per column-piece p (prefix staircase):
     * one fused 128-contraction matmul per gate-pair: rhs = xh tile
       [x(t) at parts 0:64 | h(t-1) at parts 64:128], stationary packs
       [W_x ; U_h] with the tanh-gate rows prescaled by 2.
     * sigma_ig = sigmoid(psum_ig + b_ig), sigma_fo likewise (ACT, bias AP,
       fp16 out).  s := sigmoid(2*pre_g)  => tanh(pre_g) = 2s - 1.
     * DVE (all fp16, 2x/4x modes): S=(s-.5)*4 ; Bv=S*i ; A=f*C ; C'=A+Bv
       with C := 2c, so C' = 2c'.
     * ACT: T = tanh(0.5 * C') = tanh(c') directly (same act table).
     * DVE: h = T * o written into the next step's xh window (parts 64:128).
     * Dying columns' h snapshotted into outh via copy_predicated.
"""

import numpy as np

MAXLEN = 100
B = 8192
NCORES = 8
G = B // NCORES          # graph columns per core
H = 64
F = 64
TW = 10                  # steps per time block
PIECE_BOUNDS = [0, 256, 768, 1024]   # prefix column-piece boundaries

_CACHE = {}


def _build_and_compile(schedule, weights):
    import concourse.bacc as bacc
    import concourse.mybir as mybir
    from concourse import tile

    N_t, blocks, snap = schedule
    # blocks: [(t0, nsteps, Wb, row0)]
    fp16 = mybir.dt.float16
    f32 = mybir.dt.float32
    T_end = len(N_t)
    ROWS_TOT = sum(Wb * nsteps for (_, nsteps, Wb, _) in blocks)
    MW_TOT = sum(hi - lo for t in range(len(snap)) for (_, lo, hi, _) in snap[t])
    XW = max(Wb * nsteps for (_, nsteps, Wb, _) in blocks)

    npieces = len(PIECE_BOUNDS) - 1
    PW = [PIECE_BOUNDS[p + 1] - PIECE_BOUNDS[p] for p in range(npieces)]
    psum_banks = [max(1, (2 * w * 4) // 2048) for w in PW]  # [128,2w] f32
    psum_bufs = max(1, min(2, 8 // sum(psum_banks)))

    nc = bacc.Bacc("TRN2", target_bir_lowering=False)
    xd_d = nc.dram_tensor("xd", [64, ROWS_TOT], fp16, kind="ExternalInput")
    msk_d = nc.dram_tensor("msk", [64, max(MW_TOT, 1)], mybir.dt.uint8, kind="ExternalInput")
    out_d = nc.dram_tensor("outh", [64, G], fp16, kind="ExternalOutput")
    wig_d = nc.dram_tensor("wig", [128, 128], fp16, kind="ExternalInput")
    wfo_d = nc.dram_tensor("wfo", [128, 128], fp16, kind="ExternalInput")
    brow_d = nc.dram_tensor("brow", [2, 128], fp16, kind="ExternalInput")
    sel_d = nc.dram_tensor("sel", [2, 2, 512], fp16, kind="ExternalInput")

    Sig = mybir.ActivationFunctionType.Sigmoid
    Tanh = mybir.ActivationFunctionType.Tanh
    Mult = mybir.AluOpType.mult
    Add = mybir.AluOpType.add
    Sub = mybir.AluOpType.subtract

    with tile.TileContext(nc) as tc:
        with tc.tile_pool(name="state", bufs=1) as sp, \
             tc.tile_pool(name="gates", bufs=2) as gp, \
             tc.tile_pool(name="psum", bufs=psum_bufs, space="PSUM") as pp:
            wig = sp.tile([128, 128], fp16)
            nc.sync.dma_start(out=wig, in_=wig_d.ap())
            wfo = sp.tile([128, 128], fp16)
            nc.sync.dma_start(out=wfo, in_=wfo_d.ap())
            brow2 = sp.tile([2, 128], fp16, name="brow2")
            nc.sync.dma_start(out=brow2, in_=brow_d.ap())
            sel = sp.tile([2, 2, 512], fp16, name="sel")
            nc.sync.dma_start(out=sel, in_=sel_d.ap())
            mskt_f = sp.tile([128, max(MW_TOT, 1)], mybir.dt.uint8, name="mskt")
            mskt = mskt_f[64:128, :]
            nc.sync.dma_start(out=mskt, in_=msk_d.ap())

            # persistent state (C at base partition 0, pairs with f in tt1)
            C = sp.tile([64, G], fp16, name="C")       # cell state c
            outh_f = sp.tile([128, G], fp16, name="outh")
            outh = outh_f[64:128, :]
            nc.vector.memset(C[:, :], 0.0)
            nc.vector.memset(outh, 0.0)

            # xh staging: manual double buffer, blocks alternate
            xh = [sp.tile([128, XW], fp16, name=f"xh{k}") for k in range(2)]
            # zero h-half of block0 window0 (h(-1) = 0)
            nc.vector.memset(xh[0][64:128, 0:blocks[0][2]], 0.0)

            # flatten (block, ts) -> step descriptors
            stepmap = []   # per t: (bi, win_tile_idx, win, nxt_idx, nwin)
            for bi_, (t0, nsteps, Wb, row0) in enumerate(blocks):
                for ts in range(nsteps):
                    t = t0 + ts
                    if t >= T_end:
                        continue
                    if ts + 1 < nsteps:
                        nxt_i, nwin = bi_ % 2, (ts + 1) * Wb
                    else:
                        nxt_i, nwin = (bi_ + 1) % 2, 0
                    stepmap.append((bi_, bi_ % 2, ts * Wb, nxt_i, nwin))

            def active_pieces(t):
                out = []
                if 0 <= t < T_end:
                    n = N_t[t]
                    for p in range(npieces):
                        p0, p1 = PIECE_BOUNDS[p], PIECE_BOUNDS[p + 1]
                        w = min(n, p1) - p0
                        if w > 0:
                            out.append((p, p0, w, PW[p]))
                return out

            def bias_prep(t):
                """Allocate psum tiles for step t and preload biases.
                One start=True matmul per psum bank (selector columns pick
                the f,o or 2g,i bias row)."""
                res = {}
                for (p, p0, w, Wp) in active_pieces(t):
                    tl = pp.tile([128, 2, Wp], f32, tag=f"ps{p}", name=f"ps{p}")
                    res[p] = tl
                    if 2 * Wp <= 512:
                        nc.tensor.matmul(out=tl[:, 0:2, 0:w], lhsT=brow2[:, :],
                                         rhs=sel[:, 0:2, 0:w],
                                         start=True, stop=False)
                    else:
                        nc.tensor.matmul(out=tl[:, 0, 0:w], lhsT=brow2[:, :],
                                         rhs=sel[:, 0, 0:w],
                                         start=True, stop=False)
                        nc.tensor.matmul(out=tl[:, 1, 0:w], lhsT=brow2[:, :],
                                         rhs=sel[:, 1, 0:w],
                                         start=True, stop=False)
                return res

            ps = bias_prep(0)
            dma_done = set()
            for t in range(T_end):
                bi_, xt_i, win, nxt_i, nwin = stepmap[t]
                if bi_ not in dma_done:
                    # DMA this block and the next one (double buffer)
                    for bj in (bi_, bi_ + 1):
                        if bj < len(blocks) and bj not in dma_done:
                            tb0, tns, tWb, trow0 = blocks[bj]
                            nc.sync.dma_start(
                                out=xh[bj % 2][0:64, 0:tWb * tns],
                                in_=xd_d.ap()[:, trow0:trow0 + tWb * tns])
                            dma_done.add(bj)
                xt = xh[xt_i]
                nxt = xh[nxt_i]
                pw = active_pieces(t)
                if not pw:
                    continue

                # gate matmuls (accumulate onto preloaded biases)
                for (p, p0, w, Wp) in pw:
                    rhs = xt[:, win + p0: win + p0 + w]
                    nc.tensor.matmul(out=ps[p][:, 0, 0:w], lhsT=wfo[:, :],
                                     rhs=rhs, start=False, stop=True)
                    nc.tensor.matmul(out=ps[p][:, 1, 0:w], lhsT=wig[:, :],
                                     rhs=rhs, start=False, stop=True)
                # bias prep for the NEXT step rides behind in the PE queue
                ps_next = bias_prep(t + 1)

                # per-piece chains, staggered: piece i's tanh/P5 issue
                # after piece i+1's sigma/cell ops fill the queues
                gt = {}

                def _cell(p, p0, w, Wp):
                    gt[p] = gp.tile([128, 2, Wp], fp16, tag=f"g{p}", name=f"g{p}")
                    nc.scalar.activation(out=gt[p][:, 0:2, 0:w],
                                         in_=ps[p][:, 0:2, 0:w], func=Sig)
                    # A = f * C  (f at gt[0:64, 0], C base 0) -> A base 64
                    A = gp.tile([128, 512], fp16, tag=f"A{p}", name=f"A{p}")
                    nc.vector.tensor_tensor(out=A[64:128, 0:w],
                                            in0=gt[p][0:64, 0, 0:w],
                                            in1=C[:, p0:p0 + w], op=Mult)
                    # S = (s - 0.5) * 2  (s at gt[0:64, 1]) -> base 64
                    S = gp.tile([128, 512], fp16, tag=f"S{p}", name=f"S{p}")
                    nc.vector.tensor_scalar(out=S[64:128, 0:w],
                                            in0=gt[p][0:64, 1, 0:w],
                                            scalar1=0.5, scalar2=2.0,
                                            op0=Sub, op1=Mult)
                    # Bv = S * i  (i at gt[64:128, 1])
                    Bv = gp.tile([128, 512], fp16, tag=f"B{p}", name=f"B{p}")
                    nc.vector.tensor_tensor(out=Bv[64:128, 0:w],
                                            in0=S[64:128, 0:w],
                                            in1=gt[p][64:128, 1, 0:w], op=Mult)
                    # C' = A + Bv  (both base 64; out base 0 into C)
                    nc.vector.tensor_tensor(out=C[:, p0:p0 + w],
                                            in0=A[64:128, 0:w],
                                            in1=Bv[64:128, 0:w], op=Add)

                def _tail(p, p0, w, Wp):
                    Tt = gp.tile([128, 512], fp16, tag=f"T{p}", name=f"T{p}")
                    nc.scalar.activation(out=Tt[64:128, 0:w],
                                         in_=C[:, p0:p0 + w], func=Tanh)
                    # h = T * o  (o at gt block 0 parts 64:128)
                    nc.vector.tensor_tensor(
                        out=nxt[64:128, nwin + p0: nwin + p0 + w],
                        in0=Tt[64:128, 0:w],
                        in1=gt[p][64:128, 0, 0:w], op=Mult)

                for i, pe in enumerate(pw):
                    _cell(*pe)
                    if i >= 1:
                        _tail(*pw[i - 1])
                _tail(*pw[-1])
                for (kk, lo, hi, moff) in snap[t]:
                    nc.vector.copy_predicated(
                        out=outh[:, lo:hi],
                        mask=mskt[:, moff:moff + (hi - lo)],
                        data=nxt[64:128, nwin + lo: nwin + hi])
                ps = ps_next

            nc.sync.dma_start(out=out_d.ap()[:, :], in_=outh[:, :])
    nc.compile()
    return nc


def _plan(lens):
    """Global schedule from capped lengths [B]."""
    order = np.argsort(-lens, kind="stable")
    lens_sorted = lens[order]
    T_end = int(lens_sorted.max())
    len_c = lens_sorted.reshape(G, NCORES).T  # [NCORES, G]
    t_ax = np.arange(T_end + 1)
    n_c = (len_c[:, :, None] > t_ax[None, None, :]).sum(axis=1)  # [NCORES, T+1]
    N_t = n_c.max(axis=0)
    # time blocks; Wb covers the P5 write of the previous step's width
    blocks = []
    row0 = 0
    t0 = 0
    while t0 < T_end:
        nsteps = min(TW, T_end - t0)
        Wb = int(np.ceil(N_t[max(t0 - 1, 0)] / 16) * 16)
        blocks.append((t0, nsteps, Wb, row0))
        row0 += Wb * nsteps
        t0 += nsteps
    # trailing pad block: one window for the final P5 write
    Wb_pad = int(np.ceil(N_t[T_end - 1] / 16) * 16)
    blocks.append((T_end, 1, Wb_pad, row0))
    row0 += Wb_pad

    # snapshot ranges + masks (mask marks cols whose len == t+1)
    snap = []
    moff = 0
    mask_cols = []
    for t in range(T_end):
        nt1 = n_c[:, t + 1] if t + 1 <= T_end else np.zeros(NCORES, np.int64)
        lo = int(nt1.min())
        hi = int(n_c[:, t].max())
        pieces = []
        if hi > lo:
            m = np.zeros((NCORES, hi - lo), np.uint8)
            for c in range(NCORES):
                a, b_ = int(nt1[c]), int(n_c[c, t])
                m[c, max(a - lo, 0):max(b_ - lo, 0)] = 1
            mask_cols.append(m)
            pieces.append((0, lo, hi, moff + 0))
            moff += hi - lo
        snap.append(pieces)
    masks = (np.concatenate(mask_cols, axis=1) if mask_cols
             else np.zeros((NCORES, 1), np.uint8))
    N_list = [int(x) for x in N_t[:T_end]]
    return order, len_c, n_c, N_list, blocks, snap, masks


LAST_RUN = {}


def _install_ntff_shim():
    import sys, types
    if "antenv.axon_hooks" in sys.modules:
        return
    try:
        from trn_agent_boot.trn_boot import _ntff_profile_via_ctypes
        hook = _ntff_profile_via_ctypes("/opt/axon/libaxon_pjrt.so")
    except Exception:
        hook = None
    m = types.ModuleType("antenv.axon_hooks")
    m._hook = hook
    m.get_axon_ntff_profile_hook = lambda: m._hook
    m.set_axon_ntff_profile_hook = lambda h: setattr(m, "_hook", h)
    sys.modules["antenv.axon_hooks"] = m


def kernel(x, W_ih, W_hh, b_ih, b_hh, index, dim_size, _trace=False):
    from concourse.bass_utils import run_bass_kernel_spmd
    if _trace:
        import concourse.bass_utils as _bu
        _install_ntff_shim()
        _bu.upload_artifacts = lambda d: d

    x = np.asarray(x)
    index = np.asarray(index).astype(np.int64)
    W_ih = np.asarray(W_ih, dtype=np.float32)
    W_hh = np.asarray(W_hh, dtype=np.float32)
    b_ih = np.asarray(b_ih, dtype=np.float32)
    b_hh = np.asarray(b_hh, dtype=np.float32)

    assert int(dim_size) == B, f"kernel hardcodes B={B}, got dim_size={int(dim_size)}"
    counts = np.bincount(index, minlength=B).astype(np.int64)
    offsets = np.concatenate([[0], np.cumsum(counts)[:-1]])
    lens = np.minimum(counts, MAXLEN)

    order, len_c, n_c, N_t, blocks, snap, masks = _plan(lens)

    # --- weights (torch gate order i,f,g,o) ---
    b = (b_ih + b_hh).reshape(4, H)
    Wi, Wf, Wg, Wo = W_ih.reshape(4, H, F)
    Ui, Uf, Ug, Uo = W_hh.reshape(4, H, H)
    fp16 = np.float16

    # stationary [K=128 (x 0:64, h 64:128), M=128]; ig tile = [g(x2) | i],
    # fo tile = [f | o] so tensor_tensor operand bases line up on device.
    w_ig = np.zeros((128, 128), np.float32)
    w_ig[0:64, 0:64] = 2.0 * Wg.T
    w_ig[64:128, 0:64] = 2.0 * Ug.T
    w_ig[0:64, 64:128] = Wi.T
    w_ig[64:128, 64:128] = Ui.T
    w_fo = np.zeros((128, 128), np.float32)
    w_fo[0:64, 0:64] = Wf.T
    w_fo[64:128, 0:64] = Uf.T
    w_fo[0:64, 64:128] = Wo.T
    w_fo[64:128, 64:128] = Uo.T
    w_ig = w_ig.astype(fp16)
    w_fo = w_fo.astype(fp16)
    brow = np.stack([np.concatenate([b[1], b[3]]),          # f,o biases
                     np.concatenate([2.0 * b[2], b[0]])]    # 2g,i biases
                    ).astype(fp16)                          # [2, 128]

    # --- per-core dense x slabs (step-major rows) ---
    x16 = x.astype(fp16)
    in_maps = []
    for c in range(NCORES):
        gids = order[np.arange(G) * NCORES + c]
        lens_cj = len_c[c]
        offs_cj = offsets[gids]
        parts = []
        for (t0, nsteps, Wb, row0) in blocks:
            tsl = np.arange(t0, t0 + nsteps)
            node = offs_cj[:Wb, None] + tsl[None, :]             # [Wb, nsteps]
            valid = tsl[None, :] < lens_cj[:Wb, None]
            node = np.clip(node, 0, x.shape[0] - 1)
            blk = np.where(valid[:, :, None], x16[node], fp16(0))  # [Wb,ns,64]
            parts.append(blk.transpose(1, 0, 2).reshape(nsteps * Wb, 64))
        xd = np.ascontiguousarray(np.concatenate(parts, axis=0).T)  # [64, ROWS]
        msk = np.ascontiguousarray(
            np.broadcast_to(masks[c][None, :], (64, masks.shape[1])))
        selv = np.zeros((2, 2, 512), fp16)
        selv[0, 0, :] = 1
        selv[1, 1, :] = 1
        in_maps.append({"xd": xd, "msk": msk, "wig": w_ig, "wfo": w_fo,
                        "brow": brow, "sel": selv})

    import hashlib
    key = hashlib.sha1(
        (repr((N_t, blocks, repr(snap), PIECE_BOUNDS, TW)).encode()
         + w_ig.tobytes() + w_fo.tobytes() + brow.tobytes())
    ).hexdigest()
    if key not in _CACHE:
        _CACHE[key] = _build_and_compile((N_t, blocks, snap),
                                         (w_ig, w_fo, brow))
    nc = _CACHE[key]

    res = run_bass_kernel_spmd(nc, in_maps, core_ids=list(range(NCORES)),
                               trace=_trace)
    LAST_RUN["res"] = res

    out = np.zeros((B, H), np.float32)
    for c in range(NCORES):
        hT = res.results[c]["outh"].astype(np.float32)  # [64, G]
        gids = order[np.arange(G) * NCORES + c]
        out[gids] = hT.T
    return out
